# revision 5
# baseline (speedup 1.0000x reference)
"""EGNN message-passing layer on 8 Trainium2 NeuronCores.

Strategy (dst-sharded, edge-sorted):
  * Host: sort edges by dst; shard nodes (and their incoming edges) across 8
    cores; group edges into 128-node "node tiles", chunked into 128-edge
    chunks (padded so every core runs an identical program).
  * Device, per 128-edge chunk:
      - src features: one indirect-DMA row gather from a packed [h|x] table,
        PE-transpose into [feat, edge] layout.
      - dst features: NO gather -- edges are dst-sorted, so dst rows come from
        a sequential 128-node slab via an on-chip one-hot selector matmul
        (S_T built with iota + is_equal, transposed on PE).
      - node/coord MLPs as bf16 matmuls (edge-MLP layer 2 folded into the
        node/coord layer-1 weights on the host; edge-MLP layer 1 (z) is a
        per-edge 32-vector computed on host and streamed in).
      - segment-sum by dst via selector matmul accumulating in PSUM per node
        tile; +h / +x and the b_n2*deg correction applied on-chip.
  * No collectives: each core owns its dst-node range end to end.
"""

import os
import numpy as np
import ml_dtypes

from concourse import bass, bacc, mybir, tile
from concourse.bass_utils import run_bass_kernel_spmd
from concourse.masks import make_identity

BF16 = mybir.dt.bfloat16
F32 = mybir.dt.float32
I32 = mybir.dt.int32
bf = ml_dtypes.bfloat16

N_CORES = 8
P = 128          # partitions / edges per chunk / nodes per tile
MAX_NS = 4       # subchunks per block (block = up to 512 edges)
HROW = 72        # packed table row: 64 h bf16 + 4 f32 (x,pad) as 8 bf16

_cache = {}

LAST_RESULTS = {}


def _silu(v):
    return v / (1.0 + np.exp(-v))


def _build_program(tiles_per_core, npc, ct, nchunk):
    """Build the SPMD Bass program. ct[t] = chunk count for node-tile slot t
    (same for all cores); nchunk = sum(ct)."""
    eslots = nchunk * P
    nc = bacc.Bacc("TRN2", target_bir_lowering=False, debug=False,
                   num_devices=N_CORES)

    hxb_t = nc.dram_tensor("hxb", [npc * N_CORES, HROW], BF16, kind="ExternalInput")
    slab_t = nc.dram_tensor("slab", [npc, HROW], BF16, kind="ExternalInput")
    h32_t = nc.dram_tensor("h32", [npc, 64], F32, kind="ExternalInput")
    srcidx_t = nc.dram_tensor("srcidx", [P, nchunk], I32, kind="ExternalInput")
    dstloc_t = nc.dram_tensor("dstloc", [P, nchunk], F32, kind="ExternalInput")
    zT_t = nc.dram_tensor("zT", [32, eslots], BF16, kind="ExternalInput")
    wn1a_t = nc.dram_tensor("wn1a", [128, 128], BF16, kind="ExternalInput")
    wezn_t = nc.dram_tensor("wezn", [32, 128], BF16, kind="ExternalInput")
    wc1a_t = nc.dram_tensor("wc1a", [128, 128], BF16, kind="ExternalInput")
    wezc_t = nc.dram_tensor("wezc", [32, 128], BF16, kind="ExternalInput")
    wn2_t = nc.dram_tensor("wn2", [128, 64], BF16, kind="ExternalInput")
    wc2_t = nc.dram_tensor("wc2", [128, 1], BF16, kind="ExternalInput")
    bn1_t = nc.dram_tensor("bn1", [128, 1], F32, kind="ExternalInput")
    bc1_t = nc.dram_tensor("bc1", [128, 1], F32, kind="ExternalInput")
    bn2b_t = nc.dram_tensor("bn2b", [128, 64], F32, kind="ExternalInput")

    hout_t = nc.dram_tensor("hout", [npc, 64], F32, kind="ExternalOutput")
    xout_t = nc.dram_tensor("xout", [npc, 3], F32, kind="ExternalOutput")

    SILU = mybir.ActivationFunctionType.Silu
    EQ = mybir.AluOpType.is_equal
    SUB = mybir.AluOpType.subtract
    MUL = mybir.AluOpType.mult
    ADD = mybir.AluOpType.add

    with tile.TileContext(nc) as tc:
        with tc.tile_pool(name="const", bufs=1) as cp, \
             tc.tile_pool(name="io", bufs=3) as iop, \
             tc.tile_pool(name="work", bufs=2) as wp, \
             tc.tile_pool(name="ps_tr", bufs=1, space="PSUM") as ps_tr, \
             tc.tile_pool(name="ps_st", bufs=1, space="PSUM") as ps_st, \
             tc.tile_pool(name="ps_hd", bufs=1, space="PSUM") as ps_hd, \
             tc.tile_pool(name="ps_u", bufs=1, space="PSUM") as ps_u, \
             tc.tile_pool(name="ps_v", bufs=1, space="PSUM") as ps_v, \
             tc.tile_pool(name="ps_mcw", bufs=1, space="PSUM") as ps_mcw, \
             tc.tile_pool(name="ps_agg", bufs=2, space="PSUM") as ps_agg:

            # ---- constants
            ident = cp.tile([128, 128], BF16)
            make_identity(nc, ident[:])
            iota_g = cp.tile([128, 128], BF16)
            nc.gpsimd.iota(iota_g[:], pattern=[[1, 128]], base=0,
                           channel_multiplier=0,
                           allow_small_or_imprecise_dtypes=True)
            iota_v = cp.tile([128, 128], BF16)
            nc.vector.tensor_copy(iota_v[:], iota_g[:])

            wn1a = cp.tile([128, 128], BF16)
            nc.sync.dma_start(wn1a[:], wn1a_t[:])
            wezn = cp.tile([32, 128], BF16)
            nc.sync.dma_start(wezn[:], wezn_t[:])
            wc1a = cp.tile([128, 128], BF16)
            nc.sync.dma_start(wc1a[:], wc1a_t[:])
            wezc = cp.tile([32, 128], BF16)
            nc.sync.dma_start(wezc[:], wezc_t[:])
            wn2 = cp.tile([128, 64], BF16)
            nc.sync.dma_start(wn2[:], wn2_t[:])
            wc2 = cp.tile([128, 1], BF16)
            nc.sync.dma_start(wc2[:], wc2_t[:])

            bn1_d = cp.tile([128, 1], F32)
            nc.sync.dma_start(bn1_d[:], bn1_t[:])
            bn1 = cp.tile([128, 1], F32)
            nc.scalar.copy(bn1[:], bn1_d[:])
            bc1_d = cp.tile([128, 1], F32)
            nc.sync.dma_start(bc1_d[:], bc1_t[:])
            bc1 = cp.tile([128, 1], F32)
            nc.scalar.copy(bc1[:], bc1_d[:])
            bn2b_d = cp.tile([128, 64], F32)
            nc.sync.dma_start(bn2b_d[:], bn2b_t[:])
            bn2b = cp.tile([128, 64], F32)
            nc.vector.tensor_copy(bn2b[:], bn2b_d[:])

            srcidx = cp.tile([P, nchunk], I32)
            nc.sync.dma_start(srcidx[:], srcidx_t[:])
            dstloc_d = cp.tile([P, nchunk], F32)
            nc.sync.dma_start(dstloc_d[:], dstloc_t[:])
            dstloc = cp.tile([P, nchunk], F32)
            nc.vector.tensor_copy(dstloc[:], dstloc_d[:])

            c0 = 0  # running chunk index
            for t in range(tiles_per_core):
                nct = ct[t]
                r0 = t * P
                slab_h = iop.tile([P, 64], BF16, tag="slab_h")
                nc.sync.dma_start(slab_h[:], slab_t[r0:r0 + P, 0:64])
                slab_x = iop.tile([P, 8], BF16, tag="slab_x")
                nc.sync.dma_start(slab_x[:], slab_t[r0:r0 + P, 64:72])
                slab_x4 = slab_x[:].bitcast(F32)            # [P, 4] f32
                h32 = iop.tile([P, 64], F32, tag="h32")
                nc.sync.dma_start(h32[:], h32_t[r0:r0 + P, :])

                # per-tile aggregation accumulator: its own PSUM bank --
                # an open accumulation group must not share a bank with any
                # other matmul (start=True clears has_written bank-wide).
                aggt = ps_agg.tile([128, 68], F32, space="PSUM", tag="agg")
                agg = aggt[:, :]

                nblocks = (nct + MAX_NS - 1) // MAX_NS
                for b in range(nblocks):
                    ns = min(MAX_NS, nct - b * MAX_NS)
                    W = ns * P
                    cb = c0 + b * MAX_NS      # first chunk of block
                    eo = cb * P               # first edge slot of block

                    G4 = wp.tile([P, MAX_NS * HROW], BF16, tag="G4")
                    for s in range(ns):
                        nc.gpsimd.indirect_dma_start(
                            out=G4[:, s * HROW:(s + 1) * HROW],
                            out_offset=None,
                            in_=hxb_t[:],
                            in_offset=bass.IndirectOffsetOnAxis(
                                ap=srcidx[:, cb + s:cb + s + 1], axis=0))
                    G3 = G4[:].rearrange("p (s c) -> p s c", c=HROW)

                    z_sb = wp.tile([32, MAX_NS * P], BF16, tag="z")
                    nc.sync.dma_start(z_sb[:, 0:W], zT_t[:, eo:eo + W])

                    # selector S [edge, node] (bf16) and its transpose
                    S4 = wp.tile([P, MAX_NS * P], BF16, tag="S4")
                    for s in range(ns):
                        nc.vector.tensor_tensor(
                            out=S4[:, s * P:(s + 1) * P], in0=iota_v[:],
                            in1=dstloc[:, cb + s:cb + s + 1].to_broadcast([P, P]),
                            op=EQ)
                    st_ps = ps_st.tile([128, MAX_NS * P], BF16, space="PSUM",
                                       tag="st")
                    for s in range(ns):
                        nc.tensor.transpose(out=st_ps[:, s * P:(s + 1) * P],
                                            in_=S4[:, s * P:(s + 1) * P],
                                            identity=ident[:])
                    stb = wp.tile([128, MAX_NS * P], BF16, tag="stb")
                    nc.vector.tensor_copy(stb[:, 0:W], st_ps[:, 0:W])
                    stf = wp.tile([128, MAX_NS * P], F32, tag="stf")
                    nc.scalar.copy(stf[:, 0:W], st_ps[:, 0:W])

                    # hsdT[0:64]=src h (transpose), [64:128]=dst h (selector mm)
                    tr_ps = ps_tr.tile([64, MAX_NS * P], BF16, space="PSUM",
                                       tag="tr")
                    for s in range(ns):
                        nc.tensor.transpose(out=tr_ps[:, s * P:(s + 1) * P],
                                            in_=G3[:, s, 0:64],
                                            identity=ident[:])
                    hd_ps = ps_hd.tile([128, MAX_NS * P], F32, space="PSUM",
                                       tag="hd")
                    for s in range(ns):
                        nc.tensor.matmul(hd_ps[64:128, s * P:(s + 1) * P],
                                         lhsT=slab_h[:],
                                         rhs=stb[:, s * P:(s + 1) * P],
                                         start=True, stop=True)
                    hsdT = wp.tile([128, MAX_NS * P], BF16, tag="hsdT")
                    nc.scalar.copy(hsdT[0:64, 0:W], tr_ps[:, 0:W])
                    nc.vector.tensor_copy(hsdT[64:128, 0:W], hd_ps[64:128, 0:W])

                    # per-block bank: m 0:256 | cw 256:260 | xd 260:276
                    acc = ps_mcw.tile([128, 276], F32, space="PSUM", tag="mcw")
                    for s in range(ns):
                        nc.tensor.matmul(acc[:, 260 + 4 * s:264 + 4 * s],
                                         lhsT=stf[:, s * P:(s + 1) * P],
                                         rhs=slab_x4[:, :],
                                         start=True, stop=True)

                    # node MLP layer 1 + silu
                    u_ps = ps_u.tile([128, MAX_NS * P], F32, space="PSUM",
                                     tag="u")
                    nc.tensor.matmul(u_ps[:, 0:W], lhsT=wn1a[:],
                                     rhs=hsdT[:, 0:W], start=True, stop=False)
                    nc.tensor.matmul(u_ps[:, 0:W], lhsT=wezn[:],
                                     rhs=z_sb[:, 0:W], start=False, stop=True)
                    u_sb = wp.tile([128, MAX_NS * P], BF16, tag="u_sb")
                    nc.scalar.activation(u_sb[:, 0:W], u_ps[:, 0:W], SILU,
                                         bias=bn1[:, 0:1])

                    v_ps = ps_v.tile([128, MAX_NS * P], F32, space="PSUM",
                                     tag="v")
                    nc.tensor.matmul(v_ps[:, 0:W], lhsT=wc1a[:],
                                     rhs=hsdT[:, 0:W], start=True, stop=False)
                    nc.tensor.matmul(v_ps[:, 0:W], lhsT=wezc[:],
                                     rhs=z_sb[:, 0:W], start=False, stop=True)
                    v_sb = wp.tile([128, MAX_NS * P], BF16, tag="v_sb")
                    nc.scalar.activation(v_sb[:, 0:W], v_ps[:, 0:W], SILU,
                                         bias=bc1[:, 0:1])

                    # m [edge, 64] and cw [edge, 1] per subchunk
                    for s in range(ns):
                        nc.tensor.matmul(acc[:, 64 * s:64 * (s + 1)],
                                         lhsT=u_sb[:, s * P:(s + 1) * P],
                                         rhs=wn2[:], start=True, stop=True)
                    for s in range(ns):
                        nc.tensor.matmul(acc[:, 256 + s:257 + s],
                                         lhsT=v_sb[:, s * P:(s + 1) * P],
                                         rhs=wc2[:], start=True, stop=True)

                    # agg rhs tile: [m | xw | 1] per subchunk
                    mxw = wp.tile([P, MAX_NS * 68], BF16, tag="mxw")
                    mxw3 = mxw[:].rearrange("p (s c) -> p s c", c=68)
                    for s in range(ns):
                        nc.vector.tensor_copy(
                            mxw3[:, s, 0:64], acc[:, 64 * s:64 * (s + 1)])
                    nc.vector.memset(mxw3[:, 0:ns, 67:68], 1.0)

                    # coordinate pipeline (f32)
                    xsrc = G3[:, 0:ns, 64:70].bitcast(F32)      # [P, ns, 3]
                    xd3 = acc[:, 260:260 + 4 * ns].rearrange(
                        "p (s c) -> p s c", c=4)[:, :, 0:3]
                    dirv = wp.tile([P, MAX_NS * 3], F32, tag="dir")
                    dir3 = dirv[:].rearrange("p (s c) -> p s c", c=3)
                    nc.vector.tensor_tensor(out=dir3[:, 0:ns, :], in0=xsrc,
                                            in1=xd3, op=SUB)
                    dsq = wp.tile([P, MAX_NS * 3], F32, tag="dsq")
                    nc.vector.tensor_tensor(out=dsq[:, 0:3 * ns],
                                            in0=dirv[:, 0:3 * ns],
                                            in1=dirv[:, 0:3 * ns], op=MUL)
                    len_t = wp.tile([P, MAX_NS], F32, tag="len")
                    nc.vector.tensor_reduce(
                        out=len_t[:, 0:ns],
                        in_=dsq[:].rearrange("p (s c) -> p s c", c=3)[:, 0:ns, :],
                        axis=mybir.AxisListType.X, op=ADD)
                    nc.scalar.sqrt(len_t[:, 0:ns], len_t[:, 0:ns])
                    nc.vector.tensor_scalar_max(len_t[:, 0:ns], len_t[:, 0:ns],
                                                1e-8)
                    inv_t = wp.tile([P, MAX_NS], F32, tag="inv")
                    nc.vector.reciprocal(inv_t[:, 0:ns], len_t[:, 0:ns])
                    sfac = wp.tile([P, MAX_NS], F32, tag="sfac")
                    nc.vector.tensor_tensor(out=sfac[:, 0:ns],
                                            in0=inv_t[:, 0:ns],
                                            in1=acc[:, 256:256 + ns], op=MUL)
                    nc.vector.tensor_tensor(
                        out=mxw3[:, 0:ns, 64:67], in0=dir3[:, 0:ns, :],
                        in1=sfac[:, 0:ns].unsqueeze(2).to_broadcast([P, ns, 3]),
                        op=MUL)

                    # segment-sum into per-tile aggregator
                    for s in range(ns):
                        cidx = cb + s
                        nc.tensor.matmul(agg,
                                         lhsT=S4[:, s * P:(s + 1) * P],
                                         rhs=mxw3[:, s, :],
                                         start=(cidx == c0),
                                         stop=(cidx == c0 + nct - 1))

                # ---- tile epilogue: outputs
                hout_sb = iop.tile([P, 64], F32, tag="hout")
                if nct > 0:
                    deg = iop.tile([P, 1], F32, tag="deg")
                    nc.vector.tensor_copy(deg[:], agg[:, 67:68])
                    nc.vector.tensor_tensor(out=hout_sb[:], in0=h32[:],
                                            in1=agg[:, 0:64], op=ADD)
                    degb = iop.tile([P, 64], F32, tag="degb")
                    nc.vector.tensor_scalar_mul(degb[:], bn2b[:], deg[:, 0:1])
                    nc.vector.tensor_tensor(out=hout_sb[:], in0=hout_sb[:],
                                            in1=degb[:], op=ADD)
                    xout_sb = iop.tile([P, 3], F32, tag="xout")
                    nc.vector.tensor_tensor(out=xout_sb[:], in0=slab_x4[:, 0:3],
                                            in1=agg[:, 64:67], op=ADD)
                else:
                    nc.vector.tensor_copy(hout_sb[:], h32[:])
                    xout_sb = iop.tile([P, 3], F32, tag="xout")
                    nc.vector.tensor_copy(xout_sb[:], slab_x4[:, 0:3])
                nc.sync.dma_start(hout_t[r0:r0 + P, :], hout_sb[:])
                nc.sync.dma_start(xout_t[r0:r0 + P, :], xout_sb[:])

                c0 += nct

    nc.compile()
    return nc


def kernel(h, x, edge_dist, W_e1, b_e1, W_e2, b_e2, W_n1, b_n1, W_n2, b_n2,
           W_c1, b_c1, W_c2, edge_idx):
    h = np.asarray(h, np.float32)
    x = np.asarray(x, np.float32)
    edge_dist = np.asarray(edge_dist, np.float32)
    src = np.asarray(edge_idx[0], np.int64)
    dst = np.asarray(edge_idx[1], np.int64)
    n, node_dim = h.shape
    e = src.shape[0]

    tiles_per_core = int(np.ceil(n / (N_CORES * P)))
    npc = tiles_per_core * P          # nodes per core (padded)
    n_pad = npc * N_CORES

    # ---- sort edges by dst
    order = np.argsort(dst, kind="stable")
    ssrc = src[order].astype(np.int32)
    sdst = dst[order]
    sdist = edge_dist[order]

    # ---- per-(core,tile) edge counts -> shared chunk structure
    tile_of = sdst // P                                   # global tile id
    n_tiles_global = N_CORES * tiles_per_core
    bounds = np.searchsorted(tile_of, np.arange(n_tiles_global + 1))
    cnt = (bounds[1:] - bounds[:-1]).reshape(N_CORES, tiles_per_core)
    ct = np.ceil(cnt.max(axis=0) / P).astype(np.int64)    # chunks per tile slot
    nchunk = int(ct.sum())
    eslots = nchunk * P

    # ---- per-edge z = silu(dist * W_e1 + b_e1)   [E, 32]
    z = _silu(sdist[:, None] * np.asarray(W_e1, np.float32)[0][None, :]
              + np.asarray(b_e1, np.float32)[None, :]).astype(bf)

    # ---- folded weights
    W_n1 = np.asarray(W_n1, np.float32)
    W_c1 = np.asarray(W_c1, np.float32)
    W_e2 = np.asarray(W_e2, np.float32)
    b_e2 = np.asarray(b_e2, np.float32)
    wn1a = W_n1[0:128, :].astype(bf)
    wezn = (W_e2 @ W_n1[128:160, :]).astype(bf)
    bn1p = (np.asarray(b_n1, np.float32) + b_e2 @ W_n1[128:160, :])
    wc1a = W_c1[0:128, :].astype(bf)
    wezc = (W_e2 @ W_c1[128:160, :]).astype(bf)
    bc1p = (np.asarray(b_c1, np.float32) + b_e2 @ W_c1[128:160, :])
    wn2 = np.asarray(W_n2, np.float32).astype(bf)
    wc2 = np.asarray(W_c2, np.float32).astype(bf)
    bn2b = np.tile(np.asarray(b_n2, np.float32)[None, :], (128, 1))

    # ---- packed gather table [h bf16 | x f32 | pad]
    hxb = np.zeros((n_pad, HROW), dtype=bf)
    hxb[:n, 0:64] = h.astype(bf)
    hv = hxb.view(np.uint16)
    xpad = np.zeros((n, 4), np.float32)
    xpad[:, 0:3] = x
    hv[:n, 64:72] = xpad.view(np.uint16)

    h32_pad = np.zeros((n_pad, 64), np.float32)
    h32_pad[:n] = h

    # ---- per-core padded edge arrays
    tile_first_chunk = np.zeros(tiles_per_core + 1, np.int64)
    np.cumsum(ct, out=tile_first_chunk[1:])

    in_maps = []
    for k in range(N_CORES):
        srcidx = np.zeros((eslots,), np.int32)
        dstloc = np.full((eslots,), -1.0, np.float32)
        zk = np.zeros((eslots, 32), dtype=bf)
        for t in range(tiles_per_core):
            gt = k * tiles_per_core + t
            a, bnd = bounds[gt], bounds[gt + 1]
            m = bnd - a
            o = tile_first_chunk[t] * P
            srcidx[o:o + m] = ssrc[a:bnd]
            dstloc[o:o + m] = (sdst[a:bnd] - gt * P).astype(np.float32)
            zk[o:o + m] = z[a:bnd]
        in_maps.append(dict(
            hxb=hxb,
            slab=hxb[k * npc:(k + 1) * npc],
            h32=h32_pad[k * npc:(k + 1) * npc],
            srcidx=np.ascontiguousarray(srcidx.reshape(nchunk, P).T),
            dstloc=np.ascontiguousarray(dstloc.reshape(nchunk, P).T),
            zT=np.ascontiguousarray(zk.T),
            wn1a=wn1a, wezn=wezn, wc1a=wc1a, wezc=wezc, wn2=wn2, wc2=wc2,
            bn1=bn1p[:, None].astype(np.float32),
            bc1=bc1p[:, None].astype(np.float32),
            bn2b=bn2b,
        ))

    key = (tiles_per_core, npc, tuple(ct.tolist()))
    if key not in _cache:
        _cache[key] = _build_program(tiles_per_core, npc, ct.tolist(), nchunk)
    nc = _cache[key]

    res = run_bass_kernel_spmd(nc, in_maps, core_ids=list(range(N_CORES)))
    LAST_RESULTS["exec_time_ns"] = res.exec_time_ns
    LAST_RESULTS["profile_json"] = res.profile_json
    LAST_RESULTS["nc"] = nc
    LAST_RESULTS["in_maps"] = in_maps

    h_out = np.concatenate([res.results[k]["hout"] for k in range(N_CORES)],
                           axis=0)[:n]
    x_out = np.concatenate([res.results[k]["xout"] for k in range(N_CORES)],
                           axis=0)[:n]
    return h_out, x_out
